# revision 1
# baseline (speedup 1.0000x reference)
"""GAT (2-layer, 8-head) Trainium2 kernel, 8-core SPMD.

Phase 1: head-parallel — core h computes head h's GAT layer over the full
  graph: Wh = x@W_h, e = lrelu(f1_i + f2_j), p = exp(e)*mask, out = (p@Wh)/rowsum,
  h_head = elu(out).  Uses the identity exp(lrelu(s)) = max(exp(s), exp(0.2 s))
  with s = f1_i + f2_j rank-1, so the [N,N] score matrix is built from two
  fused DVE scalar_tensor_tensor ops per 128-row tile.
Phase 2: row-parallel — host gathers h_T [512, N] (bf16), every core computes
  the full Wh2 = h@W_out, then attention + elu + log_softmax for its own
  N/8-row slice.

Host does: transposes/dtype prep, tiny weight folds (W@a vectors), gather
between the two launches. All heavy compute is on-device.
"""

import sys

for p in ("/opt/trn_rl_repo", "/opt/pypackages"):
    if p not in sys.path:
        sys.path.append(p)

import numpy as np
import ml_dtypes

import concourse.bass as bass
import concourse.bacc as bacc
import concourse.tile as tile
from concourse import mybir
from concourse.bass_utils import run_bass_kernel_spmd
from concourse.masks import make_identity

BF16 = mybir.dt.bfloat16
F32 = mybir.dt.float32
AX = mybir.AxisListType
OP = mybir.AluOpType
AF = mybir.ActivationFunctionType

N, FIN, HID, HEADS, FOUT = 4096, 512, 64, 8, 256
NCORES = 8
ALPHA = 0.2


def _flat_write_ap(t, rows, cols):
    return bass.AP(tensor=t, offset=0, ap=[[cols, rows], [1, cols]])


def _bcast_read_ap(t, parts, total):
    return bass.AP(tensor=t, offset=0, ap=[[0, parts], [1, total]])


def build_phase1(n=N, fin=FIN, hid=HID):
    """Per-core: xT [fin, n] bf16, maskT [n, n] bf16, wcat [fin, hid+2] bf16
    -> hout [hid, n] bf16 (elu'd, normalized head features, transposed)."""
    nc = bacc.Bacc("TRN2", target_bir_lowering=False, debug=False,
                   enable_asserts=False)
    kch = fin // 128          # contraction chunks for x@W
    nch = n // 128            # 128-row chunks of nodes
    nib = max(1, n // 512)    # 512-col i-blocks for PSUM banks
    ibw = min(n, 512)
    wc = hid + 2

    xT = nc.dram_tensor("xT", [fin, n], BF16, kind="ExternalInput")
    maskT = nc.dram_tensor("maskT", [n, n], BF16, kind="ExternalInput")
    wcat = nc.dram_tensor("wcat", [fin, wc], BF16, kind="ExternalInput")
    hout = nc.dram_tensor("hout", [hid, n], BF16, kind="ExternalOutput")
    scrU = nc.dram_tensor("scrU", [n], BF16)
    scrV = nc.dram_tensor("scrV", [n], BF16)
    scrR = nc.dram_tensor("scrR", [n], F32)

    with tile.TileContext(nc) as tc:
        with tc.tile_pool(name="consts", bufs=1) as consts:
            id128 = consts.tile([128, 128], F32)
            make_identity(nc, id128[:])
            wsb = consts.tile([128, kch, wc], BF16)
            for kc in range(kch):
                nc.sync.dma_start(out=wsb[:, kc, :], in_=wcat[kc * 128:(kc + 1) * 128, :])

            whall = consts.tile([128, nch, hid + 1], BF16)
            nc.vector.memset(whall[:, :, hid:hid + 1], 1.0)
            fcols = consts.tile([128, nch, 2], F32)

            # ---- Wh = x @ [W | w1 | w2] ----
            with tc.tile_pool(name="xpool", bufs=1) as xpool:
                xsb = xpool.tile([128, kch, n], BF16)
                for kc in range(kch):
                    for q4 in range(4):
                        sl = slice(q4 * (n // 4), (q4 + 1) * (n // 4))
                        nc.sync.dma_start(out=xsb[:, kc, sl],
                                          in_=xT[kc * 128:(kc + 1) * 128, sl])
                with tc.tile_pool(name="whps", bufs=4, space="PSUM") as whps:
                    for nch_i in range(nch):
                        pw = whps.tile([128, wc], F32)
                        for kc in range(kch):
                            nc.tensor.matmul(
                                out=pw[:],
                                lhsT=xsb[:, kc, nch_i * 128:(nch_i + 1) * 128],
                                rhs=wsb[:, kc, :],
                                start=(kc == 0), stop=(kc == kch - 1))
                        nc.vector.tensor_copy(out=whall[:, nch_i, 0:hid],
                                              in_=pw[:, 0:hid])
                        nc.vector.tensor_copy(out=fcols[:, nch_i, :],
                                              in_=pw[:, hid:hid + 2])

                    # ---- f1 row -> exp'd broadcast; f2 col -> exp'd scalars ----
                    u2c = consts.tile([128, nch], F32)
                    v2c = consts.tile([128, nch], F32)
                    nc.scalar.activation(out=u2c[:], in_=fcols[:, :, 1], func=AF.Exp)
                    nc.scalar.activation(out=v2c[:], in_=fcols[:, :, 1], func=AF.Exp,
                                         scale=ALPHA)
                    f1cc = consts.tile([128, nch], F32)
                    nc.scalar.activation(out=f1cc[:], in_=fcols[:, :, 0], func=AF.Copy)
                    f1T = whps.tile([nch, 128], F32, bufs=1)
                    nc.tensor.transpose(out=f1T[:], in_=f1cc[:], identity=id128[:])
                    ur = consts.tile([nch, 128], BF16)
                    vr = consts.tile([nch, 128], BF16)
                    nc.scalar.activation(out=ur[:], in_=f1T[:], func=AF.Exp)
                    nc.scalar.activation(out=vr[:], in_=f1T[:], func=AF.Exp, scale=ALPHA)
                    nc.sync.dma_start(out=_flat_write_ap(scrU, nch, 128), in_=ur[:])
                    nc.sync.dma_start(out=_flat_write_ap(scrV, nch, 128), in_=vr[:])

            u1b = consts.tile([128, n], BF16)
            v1b = consts.tile([128, n], BF16)
            for q4 in range(4):
                qn = n // 4
                for scr, dst in ((scrU, u1b), (scrV, v1b)):
                    bap = bass.AP(tensor=scr, offset=q4 * qn,
                                  ap=[[0, 128], [1, qn]])
                    nc.sync.dma_start(out=dst[:, q4 * qn:(q4 + 1) * qn], in_=bap)

            outT = consts.tile([hid + 1, n], F32)

            # ---- attention: p = max(u1b*u2[j], v1b*v2[j]) * mask ----
            with (
                tc.tile_pool(name="mpool", bufs=3) as mpool,
                tc.tile_pool(name="ampool", bufs=2) as ampool,
                tc.tile_pool(name="ppool", bufs=3) as ppool,
                tc.tile_pool(name="atps", bufs=nib, space="PSUM") as atps,
            ):
                pss = [atps.tile([hid + 1, ibw], F32, name=f"pss{_i}", tag="pss")
                       for _i in range(nib)]
                for jc in range(nch):
                    mt = mpool.tile([128, n], BF16)
                    nc.sync.dma_start(out=mt[:], in_=maskT[jc * 128:(jc + 1) * 128, :])
                    am = ampool.tile([128, n], BF16)
                    nc.scalar.activation(out=am[:], in_=u1b[:], func=AF.Copy,
                                         scale=u2c[:, jc:jc + 1])
                    bt = ampool.tile([128, n], BF16, name="bt")
                    nc.vector.tensor_scalar_mul(bt[:], v1b[:], v2c[:, jc:jc + 1])
                    qt = ampool.tile([128, n], BF16, name="qt")
                    nc.vector.tensor_max(qt[:], am[:], bt[:])
                    pt = ppool.tile([128, n], BF16)
                    nc.vector.tensor_mul(pt[:], qt[:], mt[:])
                    for ib in range(nib):
                        nc.tensor.matmul(
                            out=pss[ib][:],
                            lhsT=whall[:, jc, :],
                            rhs=pt[:, ib * ibw:(ib + 1) * ibw],
                            start=(jc == 0), stop=(jc == nch - 1))

                # ---- epilogue: normalize + elu, out h_T [hid, n] bf16 ----
                for ib in range(nib):
                    nc.vector.tensor_copy(out=outT[:, ib * ibw:(ib + 1) * ibw],
                                          in_=pss[ib][:])
            # reciprocal of rowsums in place (partition hid), bounce via DRAM
            # to broadcast across partitions (engines cannot shift partitions)
            nc.vector.reciprocal(out=outT[hid:hid + 1, :], in_=outT[hid:hid + 1, :])
            nc.sync.dma_start(out=_flat_write_ap(scrR, 1, n),
                              in_=outT[hid:hid + 1, :])
            rsb = consts.tile([hid, n], F32)
            for q4 in range(4):
                qn = n // 4
                bap = bass.AP(tensor=scrR, offset=q4 * qn, ap=[[0, hid], [1, qn]])
                nc.sync.dma_start(out=rsb[:, q4 * qn:(q4 + 1) * qn], in_=bap)
            with tc.tile_pool(name="ep1", bufs=3) as ep1:
                hv = ep1.tile([hid, n], BF16, tag="ep1")
                nc.vector.tensor_mul(hv[:], outT[0:hid, :], rsb[:])
                rp = ep1.tile([hid, n], BF16, tag="ep1", name="rp")
                nc.vector.tensor_scalar_max(rp[:], hv[:], 0.0)
                mg = ep1.tile([hid, n], BF16, tag="ep1", name="mg")
                nc.vector.tensor_scalar_min(mg[:], hv[:], 0.0)
                em = ep1.tile([hid, n], BF16, tag="ep1", name="em")
                nc.scalar.activation(out=em[:], in_=mg[:], func=AF.Exp)
                hsb = ep1.tile([hid, n], BF16, tag="ep1", name="hsb")
                nc.vector.scalar_tensor_tensor(
                    out=hsb[:], in0=em[:], scalar=-1.0, in1=rp[:],
                    op0=OP.add, op1=OP.add)
                for q4 in range(4):
                    sl = slice(q4 * (n // 4), (q4 + 1) * (n // 4))
                    nc.sync.dma_start(out=hout[:, sl], in_=hsb[:, sl])

    nc.compile()
    return nc


def build_phase2(n=N, hfull=HEADS * HID, fout=FOUT):
    """Per-core: hT [hfull, n] bf16, m2 [n, n/NCORES] bf16,
    wocat [hfull, fout+2] bf16, sel [n/128, n/(128*NCORES)] f32
    -> out [n/NCORES, fout] f32 (log_softmax rows)."""
    nc = bacc.Bacc("TRN2", target_bir_lowering=False, debug=False,
                   enable_asserts=False)
    rows = n // NCORES        # rows this core owns
    kch = hfull // 128
    nch = n // 128
    rch = rows // 128         # output 128-row chunks
    wc = fout + 2
    fb = fout // 128          # 128-col blocks of fout

    hT = nc.dram_tensor("hT", [hfull, n], BF16, kind="ExternalInput")
    m2 = nc.dram_tensor("m2", [n, rows], BF16, kind="ExternalInput")
    wocat = nc.dram_tensor("wocat", [hfull, wc], BF16, kind="ExternalInput")
    sel = nc.dram_tensor("sel", [nch, rch], F32, kind="ExternalInput")
    out = nc.dram_tensor("out", [rows, fout], F32, kind="ExternalOutput")
    scrU = nc.dram_tensor("scrU", [rows], BF16)
    scrV = nc.dram_tensor("scrV", [rows], BF16)

    with tile.TileContext(nc) as tc:
        with tc.tile_pool(name="consts", bufs=1) as consts:
            id128 = consts.tile([128, 128], F32)
            make_identity(nc, id128[:])
            hsb = consts.tile([128, kch, n], BF16)
            for kc in range(kch):
                for q4 in range(4):
                    sl = slice(q4 * (n // 4), (q4 + 1) * (n // 4))
                    nc.sync.dma_start(out=hsb[:, kc, sl],
                                      in_=hT[kc * 128:(kc + 1) * 128, sl])
            wosb = consts.tile([128, kch, wc], BF16)
            for kc in range(kch):
                nc.sync.dma_start(out=wosb[:, kc, :],
                                  in_=wocat[kc * 128:(kc + 1) * 128, :])
            selsb = consts.tile([nch, rch], F32)
            nc.sync.dma_start(out=selsb[:], in_=sel[:, :])

            woall = consts.tile([128, nch, fout + 1], BF16)
            nc.vector.memset(woall[:, :, fout:fout + 1], 1.0)
            focols = consts.tile([128, nch, 2], F32)

            # ---- Wh2 = h @ [W_out | v1o | v2o] ----
            with tc.tile_pool(name="w2ps", bufs=4, space="PSUM") as w2ps:
                for nch_i in range(nch):
                    pw = w2ps.tile([128, wc], F32)
                    for kc in range(kch):
                        nc.tensor.matmul(
                            out=pw[:],
                            lhsT=hsb[:, kc, nch_i * 128:(nch_i + 1) * 128],
                            rhs=wosb[:, kc, :],
                            start=(kc == 0), stop=(kc == kch - 1))
                    nc.vector.tensor_copy(out=woall[:, nch_i, 0:fout], in_=pw[:, 0:fout])
                    nc.vector.tensor_copy(out=focols[:, nch_i, :], in_=pw[:, fout:fout + 2])

                u2c = consts.tile([128, nch], F32)
                v2c = consts.tile([128, nch], F32)
                nc.scalar.activation(out=u2c[:], in_=focols[:, :, 1], func=AF.Exp)
                nc.scalar.activation(out=v2c[:], in_=focols[:, :, 1], func=AF.Exp,
                                     scale=ALPHA)
                f1cc = consts.tile([128, nch], F32)
                nc.scalar.activation(out=f1cc[:], in_=focols[:, :, 0], func=AF.Copy)
                f1T = w2ps.tile([nch, 128], F32, bufs=1)
                nc.tensor.transpose(out=f1T[:], in_=f1cc[:], identity=id128[:])
                f1Ts = consts.tile([nch, 128], F32)
                nc.vector.tensor_copy(out=f1Ts[:], in_=f1T[:])
                myp = w2ps.tile([rch, 128], F32, bufs=1)
                nc.tensor.matmul(out=myp[:], lhsT=selsb[:], rhs=f1Ts[:],
                                 start=True, stop=True)
                uo = consts.tile([rch, 128], BF16)
                vo = consts.tile([rch, 128], BF16)
                nc.scalar.activation(out=uo[:], in_=myp[:], func=AF.Exp)
                nc.scalar.activation(out=vo[:], in_=myp[:], func=AF.Exp, scale=ALPHA)
                nc.sync.dma_start(out=_flat_write_ap(scrU, rch, 128), in_=uo[:])
                nc.sync.dma_start(out=_flat_write_ap(scrV, rch, 128), in_=vo[:])

            u1b = consts.tile([128, rows], BF16)
            v1b = consts.tile([128, rows], BF16)
            nc.sync.dma_start(out=u1b[:], in_=_bcast_read_ap(scrU, 128, rows))
            nc.sync.dma_start(out=v1b[:], in_=_bcast_read_ap(scrV, 128, rows))

            # ---- attention over [n x rows] ----
            with (
                tc.tile_pool(name="m2pool", bufs=3) as m2pool,
                tc.tile_pool(name="am2pool", bufs=2) as am2pool,
                tc.tile_pool(name="p2pool", bufs=3) as p2pool,
                tc.tile_pool(name="sfpool", bufs=fb) as sfpool,
                tc.tile_pool(name="a2ps", bufs=fb, space="PSUM") as a2psf,
                tc.tile_pool(name="a2psr", bufs=1, space="PSUM") as a2psr,
            ):
                psf = [a2psf.tile([128, rows], F32, name=f"psf{_i}", tag="psf") for _i in range(fb)]
                psr = a2psr.tile([1, rows], F32)
                for jc in range(nch):
                    mt = m2pool.tile([128, rows], BF16)
                    nc.sync.dma_start(out=mt[:], in_=m2[jc * 128:(jc + 1) * 128, :])
                    am = am2pool.tile([128, rows], BF16)
                    nc.scalar.activation(out=am[:], in_=u1b[:], func=AF.Copy,
                                         scale=u2c[:, jc:jc + 1])
                    bt = am2pool.tile([128, rows], BF16, name="bt2")
                    nc.vector.tensor_scalar_mul(bt[:], v1b[:], v2c[:, jc:jc + 1])
                    qt = am2pool.tile([128, rows], BF16, name="qt2")
                    nc.vector.tensor_max(qt[:], am[:], bt[:])
                    pt = p2pool.tile([128, rows], BF16)
                    nc.vector.tensor_mul(pt[:], qt[:], mt[:])
                    for f in range(fb):
                        nc.tensor.matmul(
                            out=psf[f][:],
                            lhsT=woall[:, jc, f * 128:(f + 1) * 128],
                            rhs=pt[:],
                            start=(jc == 0), stop=(jc == nch - 1))
                    nc.tensor.matmul(
                        out=psr[:],
                        lhsT=woall[:, jc, fout:fout + 1],
                        rhs=pt[:],
                        start=(jc == 0), stop=(jc == nch - 1))

                sf = [sfpool.tile([128, rows], F32, name=f"sf{_i}", tag="sf") for _i in range(fb)]
                sr = consts.tile([1, rows], F32)
                for f in range(fb):
                    nc.vector.tensor_copy(out=sf[f][:], in_=psf[f][:])
                nc.vector.tensor_copy(out=sr[:], in_=psr[:])

                # ---- epilogue: per 128-row chunk transpose, norm, elu,
                # log_softmax (inside sfpool scope; attn PSUM freed first) ----
                ones11 = consts.tile([1, 1], F32)
                nc.vector.memset(ones11[:], 1.0)
                with (
                    tc.tile_pool(name="ocps", bufs=2, space="PSUM") as ocps,
                    tc.tile_pool(name="ep", bufs=3) as ep,
                ):
                    for ic in range(rch):
                        oc = ocps.tile([128, fout + 1], F32)
                        for f in range(fb):
                            nc.tensor.transpose(
                                out=oc[:, f * 128:(f + 1) * 128],
                                in_=sf[f][:, ic * 128:(ic + 1) * 128],
                                identity=id128[:])
                        nc.tensor.matmul(
                            out=oc[:, fout:fout + 1],
                            lhsT=sr[0:1, ic * 128:(ic + 1) * 128],
                            rhs=ones11[:], start=True, stop=True)
                        rc = ep.tile([128, 1], F32)
                        nc.vector.reciprocal(out=rc[:], in_=oc[:, fout:fout + 1])
                        an = ep.tile([128, fout], F32)
                        nc.vector.tensor_scalar_mul(an[:], oc[:, 0:fout], rc[:])
                        rp = ep.tile([128, fout], F32)
                        nc.vector.tensor_scalar_max(rp[:], an[:], 0.0)
                        mg = ep.tile([128, fout], F32)
                        nc.vector.tensor_scalar_min(mg[:], an[:], 0.0)
                        emt = ep.tile([128, fout], F32)
                        nc.scalar.activation(out=emt[:], in_=mg[:], func=AF.Exp)
                        elu = ep.tile([128, fout], F32)
                        nc.vector.scalar_tensor_tensor(
                            out=elu[:], in0=emt[:], scalar=-1.0, in1=rp[:],
                            op0=OP.add, op1=OP.add)
                        nmx = ep.tile([128, 1], F32)
                        nc.vector.reduce_max(out=nmx[:], in_=elu[:], axis=AX.X,
                                             negate=True)
                        ex = ep.tile([128, fout], F32)
                        sm = ep.tile([128, 1], F32)
                        nc.scalar.activation(out=ex[:], in_=elu[:], func=AF.Exp,
                                             bias=nmx[:], accum_out=sm[:])
                        lnt = ep.tile([128, 1], F32)
                        nc.scalar.activation(out=lnt[:], in_=sm[:], func=AF.Ln)
                        sh = ep.tile([128, 1], F32)
                        nc.vector.tensor_sub(sh[:], nmx[:], lnt[:])
                        fin = ep.tile([128, fout], F32)
                        nc.vector.tensor_scalar_add(fin[:], elu[:], sh[:])
                        nc.sync.dma_start(out=out[ic * 128:(ic + 1) * 128, :], in_=fin[:])

    nc.compile()
    return nc


_CACHE = {}


def _get_programs():
    if "p1" not in _CACHE:
        _CACHE["p1"] = build_phase1()
        _CACHE["p2"] = build_phase2()
    return _CACHE["p1"], _CACHE["p2"]


def kernel(x, adj, W_heads, a1_heads, a2_heads, W_out, a1_out, a2_out, **_):
    x = np.asarray(x, dtype=np.float32)
    adj = np.asarray(adj)
    W_heads = np.asarray(W_heads, dtype=np.float32)
    a1_heads = np.asarray(a1_heads, dtype=np.float32)
    a2_heads = np.asarray(a2_heads, dtype=np.float32)
    W_out = np.asarray(W_out, dtype=np.float32)
    a1_out = np.asarray(a1_out, dtype=np.float32)
    a2_out = np.asarray(a2_out, dtype=np.float32)

    bf = ml_dtypes.bfloat16
    p1, p2 = _get_programs()

    xT = np.ascontiguousarray(x.T).astype(bf)
    maskT = np.ascontiguousarray((adj > 0).T.astype(np.float32)).astype(bf)

    in1 = []
    for h in range(NCORES):
        wcat = np.concatenate(
            [W_heads[h],
             (W_heads[h] @ a1_heads[h])[:, None],
             (W_heads[h] @ a2_heads[h])[:, None]], axis=1).astype(bf)
        in1.append({"xT": xT, "maskT": maskT, "wcat": np.ascontiguousarray(wcat)})
    r1 = run_bass_kernel_spmd(p1, in1, core_ids=list(range(NCORES))).results

    hT = np.concatenate([r1[h]["hout"] for h in range(NCORES)], axis=0)
    hT = np.ascontiguousarray(hT.astype(bf))

    wocat = np.ascontiguousarray(np.concatenate(
        [W_out, (W_out @ a1_out)[:, None], (W_out @ a2_out)[:, None]],
        axis=1).astype(bf))
    rows = N // NCORES
    rch = rows // 128
    nch = N // 128
    in2 = []
    for c in range(NCORES):
        m2 = np.ascontiguousarray(maskT[:, c * rows:(c + 1) * rows])
        selm = np.zeros((nch, rch), dtype=np.float32)
        for k in range(rch):
            selm[c * rch + k, k] = 1.0
        in2.append({"hT": hT, "m2": m2, "wocat": wocat, "sel": selm})
    r2 = run_bass_kernel_spmd(p2, in2, core_ids=list(range(NCORES))).results

    out = np.concatenate([r2[c]["out"] for c in range(NCORES)], axis=0)
    return out.astype(np.float32)



# revision 3
# speedup vs baseline: 1.1981x; 1.1981x over previous
"""GAT (2-layer, 8-head) Trainium2 kernel, 8-core SPMD.

Phase 1: head-parallel — core h computes head h's GAT layer over the full
  graph.  Uses the identity  exp(lrelu(f1_i + f2_j)) = v1_i * max(w_i*u2_j, v2_j)
  with w = exp((1-a)f1), u2 = exp(f2), v2 = exp(a*f2); the v1_i factor is a
  per-row scale that cancels in the softmax normalization, so the [N,N]
  unnormalized attention is built from ONE 4x-mode tensor_scalar
  (mult+max against two per-partition scalars) and ONE 2x-mode
  tensor_tensor mask multiply per 128-row tile.
Phase 2: row-parallel — host gathers h_T [512, N] (bf16), every core computes
  the full Wh2 = h@W_out, then attention + elu + log_softmax for its own
  N/8-row slice (same max-factorization; log-softmax without max-shift since
  the logits are small).

Host does: transposes/dtype prep, tiny weight folds (W@a vectors), gather
between the two launches. All heavy compute is on-device.
"""

import sys

for p in ("/opt/trn_rl_repo", "/opt/pypackages"):
    if p not in sys.path:
        sys.path.append(p)

import numpy as np
import ml_dtypes

import concourse.bass as bass
import concourse.bacc as bacc
import concourse.tile as tile
from concourse import mybir
from concourse.bass_utils import run_bass_kernel_spmd
from concourse.masks import make_identity

BF16 = mybir.dt.bfloat16
F32 = mybir.dt.float32
AX = mybir.AxisListType
OP = mybir.AluOpType
AF = mybir.ActivationFunctionType

N, FIN, HID, HEADS, FOUT = 4096, 512, 64, 8, 256
NCORES = 8
ALPHA = 0.2


def _flat_write_ap(t, rows, cols):
    return bass.AP(tensor=t, offset=0, ap=[[cols, rows], [1, cols]])


def _bcast_read_ap(t, parts, total):
    return bass.AP(tensor=t, offset=0, ap=[[0, parts], [1, total]])


def build_phase1(n=N, fin=FIN, hid=HID):
    """Per-core: xT [fin, n] bf16, maskT [n, n] bf16, wcat [fin, hid+2] bf16
    -> hout [hid, n] bf16 (elu'd, normalized head features, transposed)."""
    nc = bacc.Bacc("TRN2", target_bir_lowering=False, debug=False,
                   enable_asserts=False)
    kch = fin // 128          # contraction chunks for x@W
    nch = n // 128            # 128-row chunks of nodes
    nib = max(1, n // 512)    # 512-col i-blocks for PSUM banks
    ibw = min(n, 512)
    wc = hid + 2

    xT = nc.dram_tensor("xT", [fin, n], BF16, kind="ExternalInput")
    maskT = nc.dram_tensor("maskT", [n, n], BF16, kind="ExternalInput")
    wcat = nc.dram_tensor("wcat", [fin, wc], BF16, kind="ExternalInput")
    hout = nc.dram_tensor("hout", [hid, n], BF16, kind="ExternalOutput")
    scrW = nc.dram_tensor("scrW", [n], BF16)
    scrR = nc.dram_tensor("scrR", [n], F32)
    scrRb = nc.dram_tensor("scrRb", [n], BF16)

    with tile.TileContext(nc) as tc:
        with tc.tile_pool(name="consts", bufs=1) as consts:
            id128 = consts.tile([128, 128], F32)
            make_identity(nc, id128[:])
            wsb = consts.tile([128, kch, wc], BF16)
            for kc in range(kch):
                nc.sync.dma_start(out=wsb[:, kc, :], in_=wcat[kc * 128:(kc + 1) * 128, :])

            whall = consts.tile([128, nch, hid + 1], BF16)
            nc.vector.memset(whall[:, :, hid:hid + 1], 1.0)
            fcols = consts.tile([128, nch, 2], F32)

            # ---- Wh = x @ [W | w1 | w2] ----
            with tc.tile_pool(name="xpool", bufs=1) as xpool:
                xsb = xpool.tile([128, kch, n], BF16)
                for kc in range(kch):
                    for q4 in range(4):
                        sl = slice(q4 * (n // 4), (q4 + 1) * (n // 4))
                        nc.sync.dma_start(out=xsb[:, kc, sl],
                                          in_=xT[kc * 128:(kc + 1) * 128, sl])
                with tc.tile_pool(name="whps", bufs=4, space="PSUM") as whps:
                    for nch_i in range(nch):
                        pw = whps.tile([128, wc], F32)
                        for kc in range(kch):
                            nc.tensor.matmul(
                                out=pw[:],
                                lhsT=xsb[:, kc, nch_i * 128:(nch_i + 1) * 128],
                                rhs=wsb[:, kc, :],
                                start=(kc == 0), stop=(kc == kch - 1))
                        nc.scalar.activation(out=whall[:, nch_i, 0:hid],
                                             in_=pw[:, 0:hid], func=AF.Copy)
                        nc.vector.tensor_copy(out=fcols[:, nch_i, :],
                                              in_=pw[:, hid:hid + 2])

                    # ---- per-j scalars u2=exp(f2), v2=exp(a f2);
                    #      per-i broadcast w = exp((1-a) f1) ----
                    u2c = consts.tile([128, nch], F32)
                    v2c = consts.tile([128, nch], F32)
                    nc.scalar.activation(out=u2c[:], in_=fcols[:, :, 1], func=AF.Exp)
                    nc.scalar.activation(out=v2c[:], in_=fcols[:, :, 1], func=AF.Exp,
                                         scale=ALPHA)
                    f1cc = consts.tile([128, nch], F32)
                    nc.scalar.activation(out=f1cc[:], in_=fcols[:, :, 0], func=AF.Copy)
                    f1T = whps.tile([nch, 128], F32, bufs=1)
                    nc.tensor.transpose(out=f1T[:], in_=f1cc[:], identity=id128[:])
                    wr = consts.tile([nch, 128], BF16)
                    nc.scalar.activation(out=wr[:], in_=f1T[:], func=AF.Exp,
                                         scale=(1.0 - ALPHA))
                    nc.sync.dma_start(out=_flat_write_ap(scrW, nch, 128), in_=wr[:])

            w1b = consts.tile([128, n], BF16)
            for q4 in range(4):
                qn = n // 4
                bap = bass.AP(tensor=scrW, offset=q4 * qn,
                              ap=[[0, 128], [1, qn]])
                nc.sync.dma_start(out=w1b[:, q4 * qn:(q4 + 1) * qn], in_=bap)

            outTb = consts.tile([hid, n], BF16)
            rrow = consts.tile([1, n], F32)

            # ---- attention: z = max(w1b*u2[j], v2[j]) * mask ----
            with (
                tc.tile_pool(name="mpool", bufs=3) as mpool,
                tc.tile_pool(name="ampool", bufs=2) as ampool,
                tc.tile_pool(name="ppool", bufs=3) as ppool,
                tc.tile_pool(name="atps", bufs=nib, space="PSUM") as atps,
            ):
                pss = [atps.tile([hid + 1, ibw], F32, name=f"pss{_i}", tag="pss")
                       for _i in range(nib)]
                for jc in range(nch):
                    mt = mpool.tile([128, n], BF16)
                    nc.sync.dma_start(out=mt[:], in_=maskT[jc * 128:(jc + 1) * 128, :])
                    mx = ampool.tile([128, n], BF16)
                    nc.vector.tensor_scalar(
                        mx[:], w1b[:], u2c[:, jc:jc + 1], v2c[:, jc:jc + 1],
                        OP.mult, OP.max)
                    pt = ppool.tile([128, n], BF16)
                    nc.vector.tensor_mul(pt[:], mx[:], mt[:])
                    for ib in range(nib):
                        nc.tensor.matmul(
                            out=pss[ib][:],
                            lhsT=whall[:, jc, :],
                            rhs=pt[:, ib * ibw:(ib + 1) * ibw],
                            start=(jc == 0), stop=(jc == nch - 1))

                # ---- epilogue: normalize + elu, out h_T [hid, n] bf16 ----
                for ib in range(nib):
                    nc.scalar.activation(
                        out=outTb[:, ib * ibw:(ib + 1) * ibw],
                        in_=pss[ib][0:hid, :], func=AF.Copy)
                    nc.vector.tensor_copy(out=rrow[:, ib * ibw:(ib + 1) * ibw],
                                          in_=pss[ib][hid:hid + 1, :])
            # reciprocal of rowsums: bounce via DRAM to reshape the [1, n]
            # row onto 128 partitions (cheap DVE recip), then broadcast
            nc.sync.dma_start(out=_flat_write_ap(scrR, 1, n), in_=rrow[:])
            r128 = consts.tile([128, nch], F32)
            nc.sync.dma_start(out=r128[:],
                              in_=bass.AP(tensor=scrR, offset=0,
                                          ap=[[nch, 128], [1, nch]]))
            rb128 = consts.tile([128, nch], BF16)
            with nc.allow_low_precision(reason="softmax denom reciprocal to bf16"):
                nc.vector.reciprocal(out=rb128[:], in_=r128[:])
            nc.sync.dma_start(out=_flat_write_ap(scrRb, 128, nch), in_=rb128[:])
            rsb = consts.tile([hid, n], BF16)
            for q4 in range(4):
                qn = n // 4
                bap = bass.AP(tensor=scrRb, offset=q4 * qn, ap=[[0, hid], [1, qn]])
                nc.sync.dma_start(out=rsb[:, q4 * qn:(q4 + 1) * qn], in_=bap)
            with tc.tile_pool(name="ep1", bufs=3) as ep1:
                hv = ep1.tile([hid, n], BF16, tag="ep1")
                nc.vector.tensor_mul(hv[:], outTb[:], rsb[:])
                rp = ep1.tile([hid, n], BF16, tag="ep1", name="rp")
                nc.vector.tensor_scalar_max(rp[:], hv[:], 0.0)
                em = ep1.tile([hid, n], BF16, tag="ep1", name="em")
                nc.scalar.activation(out=em[:], in_=hv[:], func=AF.Exp)
                mn = ep1.tile([hid, n], BF16, tag="ep1", name="mn")
                nc.vector.tensor_scalar(mn[:], em[:], 1.0, -1.0, OP.min, OP.add)
                hsb = ep1.tile([hid, n], BF16, tag="ep1", name="hsb")
                nc.vector.tensor_add(hsb[:], mn[:], rp[:])
                for q4 in range(4):
                    sl = slice(q4 * (n // 4), (q4 + 1) * (n // 4))
                    nc.sync.dma_start(out=hout[:, sl], in_=hsb[:, sl])

    nc.compile()
    return nc


def build_phase2(n=N, hfull=HEADS * HID, fout=FOUT):
    """Per-core: hT [hfull, n] bf16, m2 [n, n/NCORES] bf16,
    wocat [hfull, fout+2] bf16, sel [n/128, n/(128*NCORES)] f32
    -> out [n/NCORES, fout] f32 (log_softmax rows)."""
    nc = bacc.Bacc("TRN2", target_bir_lowering=False, debug=False,
                   enable_asserts=False)
    rows = n // NCORES        # rows this core owns
    kch = hfull // 128
    nch = n // 128
    rch = rows // 128         # output 128-row chunks
    wc = fout + 2
    fb = fout // 128          # 128-col blocks of fout

    hT = nc.dram_tensor("hT", [hfull, n], BF16, kind="ExternalInput")
    m2 = nc.dram_tensor("m2", [n, rows], BF16, kind="ExternalInput")
    wocat = nc.dram_tensor("wocat", [hfull, wc], BF16, kind="ExternalInput")
    sel = nc.dram_tensor("sel", [nch, rch], F32, kind="ExternalInput")
    out = nc.dram_tensor("out", [rows, fout], F32, kind="ExternalOutput")
    scrW = nc.dram_tensor("scrW", [rows], BF16)

    with tile.TileContext(nc) as tc:
        with tc.tile_pool(name="consts", bufs=1) as consts:
            id128 = consts.tile([128, 128], F32)
            make_identity(nc, id128[:])
            hsb = consts.tile([128, kch, n], BF16)
            for kc in range(kch):
                for q4 in range(4):
                    sl = slice(q4 * (n // 4), (q4 + 1) * (n // 4))
                    nc.sync.dma_start(out=hsb[:, kc, sl],
                                      in_=hT[kc * 128:(kc + 1) * 128, sl])
            wosb = consts.tile([128, kch, wc], BF16)
            for kc in range(kch):
                nc.sync.dma_start(out=wosb[:, kc, :],
                                  in_=wocat[kc * 128:(kc + 1) * 128, :])
            selsb = consts.tile([nch, rch], F32)
            nc.sync.dma_start(out=selsb[:], in_=sel[:, :])

            woall = consts.tile([128, nch, fout + 1], BF16)
            nc.vector.memset(woall[:, :, fout:fout + 1], 1.0)
            focols = consts.tile([128, nch, 2], F32)

            # ---- Wh2 = h @ [W_out | v1o | v2o] ----
            with tc.tile_pool(name="w2ps", bufs=4, space="PSUM") as w2ps:
                for nch_i in range(nch):
                    pw = w2ps.tile([128, wc], F32)
                    for kc in range(kch):
                        nc.tensor.matmul(
                            out=pw[:],
                            lhsT=hsb[:, kc, nch_i * 128:(nch_i + 1) * 128],
                            rhs=wosb[:, kc, :],
                            start=(kc == 0), stop=(kc == kch - 1))
                    nc.scalar.activation(out=woall[:, nch_i, 0:fout],
                                         in_=pw[:, 0:fout], func=AF.Copy)
                    nc.vector.tensor_copy(out=focols[:, nch_i, :], in_=pw[:, fout:fout + 2])

                u2c = consts.tile([128, nch], F32)
                v2c = consts.tile([128, nch], F32)
                nc.scalar.activation(out=u2c[:], in_=focols[:, :, 1], func=AF.Exp)
                nc.scalar.activation(out=v2c[:], in_=focols[:, :, 1], func=AF.Exp,
                                     scale=ALPHA)
                f1cc = consts.tile([128, nch], F32)
                nc.scalar.activation(out=f1cc[:], in_=focols[:, :, 0], func=AF.Copy)
                f1T = w2ps.tile([nch, 128], F32, bufs=1)
                nc.tensor.transpose(out=f1T[:], in_=f1cc[:], identity=id128[:])
                f1Ts = consts.tile([nch, 128], F32)
                nc.vector.tensor_copy(out=f1Ts[:], in_=f1T[:])
                myp = w2ps.tile([rch, 128], F32, bufs=1)
                nc.tensor.matmul(out=myp[:], lhsT=selsb[:], rhs=f1Ts[:],
                                 start=True, stop=True)
                wo = consts.tile([rch, 128], BF16)
                nc.scalar.activation(out=wo[:], in_=myp[:], func=AF.Exp,
                                     scale=(1.0 - ALPHA))
                nc.sync.dma_start(out=_flat_write_ap(scrW, rch, 128), in_=wo[:])

            w2b = consts.tile([128, rows], BF16)
            nc.sync.dma_start(out=w2b[:], in_=_bcast_read_ap(scrW, 128, rows))

            # ---- attention over [n x rows] ----
            with (
                tc.tile_pool(name="m2pool", bufs=3) as m2pool,
                tc.tile_pool(name="am2pool", bufs=2) as am2pool,
                tc.tile_pool(name="p2pool", bufs=3) as p2pool,
                tc.tile_pool(name="sfpool", bufs=fb) as sfpool,
                tc.tile_pool(name="a2ps", bufs=fb, space="PSUM") as a2psf,
                tc.tile_pool(name="a2psr", bufs=1, space="PSUM") as a2psr,
            ):
                psf = [a2psf.tile([128, rows], F32, name=f"psf{_i}", tag="psf") for _i in range(fb)]
                psr = a2psr.tile([1, rows], F32)
                for jc in range(nch):
                    mt = m2pool.tile([128, rows], BF16)
                    nc.sync.dma_start(out=mt[:], in_=m2[jc * 128:(jc + 1) * 128, :])
                    mx = am2pool.tile([128, rows], BF16)
                    nc.vector.tensor_scalar(
                        mx[:], w2b[:], u2c[:, jc:jc + 1], v2c[:, jc:jc + 1],
                        OP.mult, OP.max)
                    pt = p2pool.tile([128, rows], BF16)
                    nc.vector.tensor_mul(pt[:], mx[:], mt[:])
                    for f in range(fb):
                        nc.tensor.matmul(
                            out=psf[f][:],
                            lhsT=woall[:, jc, f * 128:(f + 1) * 128],
                            rhs=pt[:],
                            start=(jc == 0), stop=(jc == nch - 1))
                    nc.tensor.matmul(
                        out=psr[:],
                        lhsT=woall[:, jc, fout:fout + 1],
                        rhs=pt[:],
                        start=(jc == 0), stop=(jc == nch - 1))

                sf = [sfpool.tile([128, rows], F32, name=f"sf{_i}", tag="sf") for _i in range(fb)]
                sr = consts.tile([1, rows], F32)
                for f in range(fb):
                    nc.vector.tensor_copy(out=sf[f][:], in_=psf[f][:])
                nc.vector.tensor_copy(out=sr[:], in_=psr[:])

                # ---- epilogue: per 128-row chunk transpose, norm, elu;
                # log_softmax without max-shift (logits are small) and with
                # Exp/Ln batched to avoid act-table thrash ----
                ones11 = consts.tile([1, 1], F32)
                nc.vector.memset(ones11[:], 1.0)
                helu = consts.tile([128, rch, fout], BF16)
                smv = consts.tile([128, rch], F32)
                with (
                    tc.tile_pool(name="ocps", bufs=2, space="PSUM") as ocps,
                    tc.tile_pool(name="ep", bufs=3) as ep,
                ):
                    for ic in range(rch):
                        oc = ocps.tile([128, fout + 1], F32)
                        for f in range(fb):
                            nc.tensor.transpose(
                                out=oc[:, f * 128:(f + 1) * 128],
                                in_=sf[f][:, ic * 128:(ic + 1) * 128],
                                identity=id128[:])
                        nc.tensor.matmul(
                            out=oc[:, fout:fout + 1],
                            lhsT=sr[0:1, ic * 128:(ic + 1) * 128],
                            rhs=ones11[:], start=True, stop=True)
                        rc = ep.tile([128, 1], F32)
                        nc.vector.reciprocal(out=rc[:], in_=oc[:, fout:fout + 1])
                        an = ep.tile([128, fout], BF16)
                        nc.vector.tensor_scalar_mul(an[:], oc[:, 0:fout], rc[:])
                        rp = ep.tile([128, fout], BF16)
                        nc.vector.tensor_scalar_max(rp[:], an[:], 0.0)
                        emt = ep.tile([128, fout], BF16)
                        nc.scalar.activation(out=emt[:], in_=an[:], func=AF.Exp)
                        mn = ep.tile([128, fout], BF16)
                        nc.vector.tensor_scalar(mn[:], emt[:], 1.0, -1.0,
                                                OP.min, OP.add)
                        nc.vector.tensor_add(helu[:, ic, :], mn[:], rp[:])
                        ex2 = ep.tile([128, fout], BF16)
                        nc.scalar.activation(out=ex2[:], in_=helu[:, ic, :],
                                             func=AF.Exp,
                                             accum_out=smv[:, ic:ic + 1])
                    lnv = consts.tile([128, rch], F32)
                    nc.scalar.activation(out=lnv[:], in_=smv[:], func=AF.Ln)
                    for ic in range(rch):
                        fin = ep.tile([128, fout], F32, name="fin")
                        nc.vector.tensor_scalar_sub(fin[:], helu[:, ic, :],
                                                    lnv[:, ic:ic + 1])
                        nc.sync.dma_start(out=out[ic * 128:(ic + 1) * 128, :], in_=fin[:])

    nc.compile()
    return nc


_CACHE = {}


def _get_programs():
    if "p1" not in _CACHE:
        _CACHE["p1"] = build_phase1()
        _CACHE["p2"] = build_phase2()
    return _CACHE["p1"], _CACHE["p2"]


def kernel(x, adj, W_heads, a1_heads, a2_heads, W_out, a1_out, a2_out, **_):
    x = np.asarray(x, dtype=np.float32)
    adj = np.asarray(adj)
    W_heads = np.asarray(W_heads, dtype=np.float32)
    a1_heads = np.asarray(a1_heads, dtype=np.float32)
    a2_heads = np.asarray(a2_heads, dtype=np.float32)
    W_out = np.asarray(W_out, dtype=np.float32)
    a1_out = np.asarray(a1_out, dtype=np.float32)
    a2_out = np.asarray(a2_out, dtype=np.float32)

    bf = ml_dtypes.bfloat16
    p1, p2 = _get_programs()

    xT = np.ascontiguousarray(x.T).astype(bf)
    maskT = np.ascontiguousarray((adj > 0).T.astype(np.float32)).astype(bf)

    in1 = []
    for h in range(NCORES):
        wcat = np.concatenate(
            [W_heads[h],
             (W_heads[h] @ a1_heads[h])[:, None],
             (W_heads[h] @ a2_heads[h])[:, None]], axis=1).astype(bf)
        in1.append({"xT": xT, "maskT": maskT, "wcat": np.ascontiguousarray(wcat)})
    r1 = run_bass_kernel_spmd(p1, in1, core_ids=list(range(NCORES))).results

    hT = np.concatenate([r1[h]["hout"] for h in range(NCORES)], axis=0)
    hT = np.ascontiguousarray(hT.astype(bf))

    wocat = np.ascontiguousarray(np.concatenate(
        [W_out, (W_out @ a1_out)[:, None], (W_out @ a2_out)[:, None]],
        axis=1).astype(bf))
    rows = N // NCORES
    rch = rows // 128
    nch = N // 128
    in2 = []
    for c in range(NCORES):
        m2 = np.ascontiguousarray(maskT[:, c * rows:(c + 1) * rows])
        selm = np.zeros((nch, rch), dtype=np.float32)
        for k in range(rch):
            selm[c * rch + k, k] = 1.0
        in2.append({"hT": hT, "m2": m2, "wocat": wocat, "sel": selm})
    r2 = run_bass_kernel_spmd(p2, in2, core_ids=list(range(NCORES))).results

    out = np.concatenate([r2[c]["out"] for c in range(NCORES)], axis=0)
    return out.astype(np.float32)


# revision 4
# speedup vs baseline: 1.2016x; 1.0029x over previous
"""GAT (2-layer, 8-head) Trainium2 kernel, 8-core SPMD.

Phase 1: head-parallel — core h computes head h's GAT layer over the full
  graph.  Uses the identity  exp(lrelu(f1_i + f2_j)) = v1_i * max(w_i*u2_j, v2_j)
  with w = exp((1-a)f1), u2 = exp(f2), v2 = exp(a*f2); the v1_i factor is a
  per-row scale that cancels in the softmax normalization, so the [N,N]
  unnormalized attention needs ONE 4x-mode tensor_scalar (mult+max against
  two per-partition scalars) and ONE 2x-mode tensor_tensor mask multiply per
  128-row tile (mask multiply split between DVE and GpSimd).  The tiny f1/f2
  vectors (x @ (W@a), O(N*F)) are folded on the host like the W@a folds, so
  the attention stream starts immediately.  A zero-weight dummy matmul gates
  each 4-tile block of PE work so the tensor engine runs in long bursts and
  ramps out of its low-power state.
Phase 2: row-parallel — host gathers h_T [512, N] (bf16), every core computes
  the full Wh2 = h@W_out chunk-by-chunk interleaved with its attention
  matmuls (keeps PE continuously busy), then elu + log_softmax (no max-shift;
  logits are small) for its own N/8-row slice.
"""

import sys

for p in ("/opt/trn_rl_repo", "/opt/pypackages"):
    if p not in sys.path:
        sys.path.append(p)

import numpy as np
import ml_dtypes

import concourse.bass as bass
import concourse.bacc as bacc
import concourse.tile as tile
from concourse import mybir
from concourse.bass_utils import run_bass_kernel_spmd
from concourse.masks import make_identity

BF16 = mybir.dt.bfloat16
F32 = mybir.dt.float32
AX = mybir.AxisListType
OP = mybir.AluOpType
AF = mybir.ActivationFunctionType

N, FIN, HID, HEADS, FOUT = 4096, 512, 64, 8, 256
NCORES = 8
ALPHA = 0.2
CV = 2560      # columns of the mask-multiply done on DVE; rest on GpSimd
BK = 4         # attention tiles per PE burst block (phase 1)


def _flat_write_ap(t, rows, cols):
    return bass.AP(tensor=t, offset=0, ap=[[cols, rows], [1, cols]])


def build_phase1(n=N, fin=FIN, hid=HID):
    """Per-core: xT [fin, n] bf16, maskT [n, n] bf16, wcat [fin, hid] bf16,
    wvec [n] bf16 (= exp((1-a)f1)), u2i/v2i [128, n/128] f32
    -> hout [hid, n] bf16 (elu'd, normalized head features, transposed)."""
    nc = bacc.Bacc("TRN2", target_bir_lowering=False, debug=False,
                   enable_asserts=False)
    kch = fin // 128          # contraction chunks for x@W
    nch = n // 128            # 128-row chunks of nodes
    nib = max(1, n // 512)    # 512-col i-blocks for PSUM banks
    ibw = min(n, 512)

    xT = nc.dram_tensor("xT", [fin, n], BF16, kind="ExternalInput")
    maskT = nc.dram_tensor("maskT", [n, n], BF16, kind="ExternalInput")
    wcat = nc.dram_tensor("wcat", [fin, hid], BF16, kind="ExternalInput")
    wvec = nc.dram_tensor("wvec", [n], BF16, kind="ExternalInput")
    u2i = nc.dram_tensor("u2i", [128, nch], F32, kind="ExternalInput")
    v2i = nc.dram_tensor("v2i", [128, nch], F32, kind="ExternalInput")
    hout = nc.dram_tensor("hout", [hid, n], BF16, kind="ExternalOutput")
    scrR = nc.dram_tensor("scrR", [n], BF16)
    scrRb = nc.dram_tensor("scrRb", [n], BF16)

    with tile.TileContext(nc) as tc:
        with tc.tile_pool(name="consts", bufs=1) as consts:
            wsb = consts.tile([128, kch, hid], BF16)
            for kc in range(kch):
                nc.sync.dma_start(out=wsb[:, kc, :], in_=wcat[kc * 128:(kc + 1) * 128, :])
            u2c = consts.tile([128, nch], F32)
            v2c = consts.tile([128, nch], F32)
            nc.sync.dma_start(out=u2c[:], in_=u2i[:, :])
            nc.sync.dma_start(out=v2c[:], in_=v2i[:, :])
            w1b = consts.tile([128, n], BF16)
            for q4 in range(4):
                qn = n // 4
                bap = bass.AP(tensor=wvec, offset=q4 * qn, ap=[[0, 128], [1, qn]])
                nc.sync.dma_start(out=w1b[:, q4 * qn:(q4 + 1) * qn], in_=bap)
            zcol = consts.tile([128, 1], BF16)
            nc.vector.memset(zcol[:], 0.0)

            whall = consts.tile([128, nch, hid + 1], BF16)
            nc.vector.memset(whall[:, :, hid:hid + 1], 1.0)

            # ---- Wh = x @ W  (column-major loads so chunk 0 is ready fast) ----
            with tc.tile_pool(name="xpool", bufs=1) as xpool:
                xsb = xpool.tile([128, kch, n], BF16)
                for q8 in range(8):
                    sl = slice(q8 * (n // 8), (q8 + 1) * (n // 8))
                    for kc in range(kch):
                        nc.sync.dma_start(out=xsb[:, kc, sl],
                                          in_=xT[kc * 128:(kc + 1) * 128, sl])
                with tc.tile_pool(name="whps", bufs=4, space="PSUM") as whps:
                    for nch_i in range(nch):
                        pw = whps.tile([128, hid], F32)
                        for kc in range(kch):
                            nc.tensor.matmul(
                                out=pw[:],
                                lhsT=xsb[:, kc, nch_i * 128:(nch_i + 1) * 128],
                                rhs=wsb[:, kc, :],
                                start=(kc == 0), stop=(kc == kch - 1))
                        nc.scalar.activation(out=whall[:, nch_i, 0:hid],
                                             in_=pw[:], func=AF.Copy)

            outTb = consts.tile([hid + 1, n], BF16)

            # ---- attention: z = max(w1b*u2[j], v2[j]) * mask ----
            with (
                tc.tile_pool(name="mpool", bufs=6) as mpool,
                tc.tile_pool(name="ampool", bufs=3) as ampool,
                tc.tile_pool(name="ppool", bufs=BK + 2) as ppool,
                tc.tile_pool(name="atps", bufs=nib, space="PSUM") as atps,
            ):
                pss = [atps.tile([hid + 1, ibw], F32, name=f"pss{_i}", tag="pss")
                       for _i in range(nib)]
                for jb in range(nch // BK):
                    pts = []
                    for k in range(BK):
                        jc = jb * BK + k
                        mt = mpool.tile([128, n], BF16)
                        nc.sync.dma_start(out=mt[:], in_=maskT[jc * 128:(jc + 1) * 128, :])
                        mx = ampool.tile([128, n], BF16)
                        nc.vector.tensor_scalar(
                            mx[:], w1b[:], u2c[:, jc:jc + 1], v2c[:, jc:jc + 1],
                            OP.mult, OP.max)
                        pt = ppool.tile([128, n], BF16)
                        nc.vector.tensor_mul(pt[:, 0:CV], mx[:, 0:CV], mt[:, 0:CV])
                        nc.gpsimd.tensor_mul(pt[:, CV:n], mx[:, CV:n], mt[:, CV:n])
                        pts.append(pt)
                    if jb > 0:
                        # zero-weight gate: PE waits for the block's last pt,
                        # then bursts 4 tiles of matmuls back-to-back (ramps
                        # the PE clock out of its low-power state)
                        nc.tensor.matmul(
                            out=pss[0][0:1, 0:1], lhsT=zcol[:],
                            rhs=pts[BK - 1][:, 0:1],
                            start=False, stop=False, skip_group_check=True)
                    for k in range(BK):
                        jc = jb * BK + k
                        for ib in range(nib):
                            nc.tensor.matmul(
                                out=pss[ib][:],
                                lhsT=whall[:, jc, :],
                                rhs=pts[k][:, ib * ibw:(ib + 1) * ibw],
                                start=(jc == 0), stop=(jc == nch - 1))

                # ---- epilogue: normalize + elu, out h_T [hid, n] bf16 ----
                for ib in range(nib):
                    nc.scalar.activation(
                        out=outTb[:, ib * ibw:(ib + 1) * ibw],
                        in_=pss[ib][:], func=AF.Copy)
            # reciprocal of rowsums: bounce via DRAM to reshape the [1, n]
            # row onto 128 partitions (cheap DVE recip), then broadcast
            nc.sync.dma_start(out=_flat_write_ap(scrR, 1, n),
                              in_=outTb[hid:hid + 1, :])
            r128 = consts.tile([128, nch], BF16)
            nc.sync.dma_start(out=r128[:],
                              in_=bass.AP(tensor=scrR, offset=0,
                                          ap=[[nch, 128], [1, nch]]))
            rb128 = consts.tile([128, nch], BF16)
            with nc.allow_low_precision(reason="softmax denom reciprocal to bf16"):
                nc.vector.reciprocal(out=rb128[:], in_=r128[:])
            nc.sync.dma_start(out=_flat_write_ap(scrRb, 128, nch), in_=rb128[:])
            rsb = consts.tile([hid, n], BF16)
            for q4 in range(4):
                qn = n // 4
                bap = bass.AP(tensor=scrRb, offset=q4 * qn, ap=[[0, hid], [1, qn]])
                nc.sync.dma_start(out=rsb[:, q4 * qn:(q4 + 1) * qn], in_=bap)
            with tc.tile_pool(name="ep1", bufs=3) as ep1:
                hv = ep1.tile([hid, n], BF16, tag="ep1")
                nc.vector.tensor_mul(hv[:], outTb[0:hid, :], rsb[:])
                rp = ep1.tile([hid, n], BF16, tag="ep1", name="rp")
                nc.vector.tensor_scalar_max(rp[:], hv[:], 0.0)
                em = ep1.tile([hid, n], BF16, tag="ep1", name="em")
                nc.scalar.activation(out=em[:], in_=hv[:], func=AF.Exp)
                mn = ep1.tile([hid, n], BF16, tag="ep1", name="mn")
                nc.vector.tensor_scalar(mn[:], em[:], 1.0, -1.0, OP.min, OP.add)
                hsb = ep1.tile([hid, n], BF16, tag="ep1", name="hsb")
                nc.vector.tensor_add(hsb[:], mn[:], rp[:])
                for q4 in range(4):
                    sl = slice(q4 * (n // 4), (q4 + 1) * (n // 4))
                    nc.sync.dma_start(out=hout[:, sl], in_=hsb[:, sl])

    nc.compile()
    return nc


def build_phase2(n=N, hfull=HEADS * HID, fout=FOUT):
    """Per-core: hT [hfull, n] bf16, m2 [n, n/NCORES] bf16,
    wocat [hfull, fout] bf16, wrow [n/NCORES] bf16, u2i/v2i [128, n/128] f32
    -> out [n/NCORES, fout] f32 (log_softmax rows)."""
    nc = bacc.Bacc("TRN2", target_bir_lowering=False, debug=False,
                   enable_asserts=False)
    rows = n // NCORES        # rows this core owns
    kch = hfull // 128
    nch = n // 128
    rch = rows // 128         # output 128-row chunks
    fb = fout // 128          # 128-col blocks of fout

    hT = nc.dram_tensor("hT", [hfull, n], BF16, kind="ExternalInput")
    m2 = nc.dram_tensor("m2", [n, rows], BF16, kind="ExternalInput")
    wocat = nc.dram_tensor("wocat", [hfull, fout], BF16, kind="ExternalInput")
    wrow = nc.dram_tensor("wrow", [rows], BF16, kind="ExternalInput")
    u2i = nc.dram_tensor("u2i", [128, nch], F32, kind="ExternalInput")
    v2i = nc.dram_tensor("v2i", [128, nch], F32, kind="ExternalInput")
    out = nc.dram_tensor("out", [rows, fout], F32, kind="ExternalOutput")

    with tile.TileContext(nc) as tc:
        with tc.tile_pool(name="consts", bufs=1) as consts:
            id128 = consts.tile([128, 128], F32)
            make_identity(nc, id128[:])
            wosb = consts.tile([128, kch, fout], BF16)
            for kc in range(kch):
                nc.sync.dma_start(out=wosb[:, kc, :],
                                  in_=wocat[kc * 128:(kc + 1) * 128, :])
            u2c = consts.tile([128, nch], F32)
            v2c = consts.tile([128, nch], F32)
            nc.sync.dma_start(out=u2c[:], in_=u2i[:, :])
            nc.sync.dma_start(out=v2c[:], in_=v2i[:, :])
            w2b = consts.tile([128, rows], BF16)
            nc.sync.dma_start(out=w2b[:],
                              in_=bass.AP(tensor=wrow, offset=0,
                                          ap=[[0, 128], [1, rows]]))
            hsb = consts.tile([128, kch, n], BF16)
            for q8 in range(8):
                sl = slice(q8 * (n // 8), (q8 + 1) * (n // 8))
                for kc in range(kch):
                    nc.sync.dma_start(out=hsb[:, kc, sl],
                                      in_=hT[kc * 128:(kc + 1) * 128, sl])

            woall = consts.tile([128, nch, fout + 1], BF16)
            nc.vector.memset(woall[:, :, fout:fout + 1], 1.0)

            # ---- interleaved: Wh2 chunk jc, then attention tile jc ----
            with (
                tc.tile_pool(name="w2ps", bufs=2, space="PSUM") as w2ps,
                tc.tile_pool(name="m2pool", bufs=4) as m2pool,
                tc.tile_pool(name="am2pool", bufs=3) as am2pool,
                tc.tile_pool(name="p2pool", bufs=4) as p2pool,
                tc.tile_pool(name="sfpool", bufs=fb) as sfpool,
                tc.tile_pool(name="a2ps", bufs=fb, space="PSUM") as a2psf,
                tc.tile_pool(name="a2psr", bufs=1, space="PSUM") as a2psr,
            ):
                psf = [a2psf.tile([128, rows], F32, name=f"psf{_i}", tag="psf") for _i in range(fb)]
                psr = a2psr.tile([1, rows], F32)
                for jc in range(nch):
                    pw = w2ps.tile([128, fout], F32)
                    for kc in range(kch):
                        nc.tensor.matmul(
                            out=pw[:],
                            lhsT=hsb[:, kc, jc * 128:(jc + 1) * 128],
                            rhs=wosb[:, kc, :],
                            start=(kc == 0), stop=(kc == kch - 1))
                    nc.scalar.activation(out=woall[:, jc, 0:fout],
                                         in_=pw[:], func=AF.Copy)

                    mt = m2pool.tile([128, rows], BF16)
                    nc.sync.dma_start(out=mt[:], in_=m2[jc * 128:(jc + 1) * 128, :])
                    mx = am2pool.tile([128, rows], BF16)
                    nc.vector.tensor_scalar(
                        mx[:], w2b[:], u2c[:, jc:jc + 1], v2c[:, jc:jc + 1],
                        OP.mult, OP.max)
                    pt = p2pool.tile([128, rows], BF16)
                    nc.vector.tensor_mul(pt[:], mx[:], mt[:])
                    for f in range(fb):
                        nc.tensor.matmul(
                            out=psf[f][:],
                            lhsT=woall[:, jc, f * 128:(f + 1) * 128],
                            rhs=pt[:],
                            start=(jc == 0), stop=(jc == nch - 1))
                    nc.tensor.matmul(
                        out=psr[:],
                        lhsT=woall[:, jc, fout:fout + 1],
                        rhs=pt[:],
                        start=(jc == 0), stop=(jc == nch - 1))

                sf = [sfpool.tile([128, rows], F32, name=f"sf{_i}", tag="sf") for _i in range(fb)]
                sr = consts.tile([1, rows], F32)
                for f in range(fb):
                    nc.vector.tensor_copy(out=sf[f][:], in_=psf[f][:])
                nc.vector.tensor_copy(out=sr[:], in_=psr[:])

                # ---- epilogue: per 128-row chunk transpose, norm, elu;
                # log_softmax without max-shift (logits are small) and with
                # Exp/Ln batched to avoid act-table thrash ----
                ones11 = consts.tile([1, 1], F32)
                nc.vector.memset(ones11[:], 1.0)
                helu = consts.tile([128, rch, fout], BF16)
                smv = consts.tile([128, rch], F32)
                with (
                    tc.tile_pool(name="ocps", bufs=2, space="PSUM") as ocps,
                    tc.tile_pool(name="ep", bufs=3) as ep,
                ):
                    for ic in range(rch):
                        oc = ocps.tile([128, fout + 1], F32)
                        for f in range(fb):
                            nc.tensor.transpose(
                                out=oc[:, f * 128:(f + 1) * 128],
                                in_=sf[f][:, ic * 128:(ic + 1) * 128],
                                identity=id128[:])
                        nc.tensor.matmul(
                            out=oc[:, fout:fout + 1],
                            lhsT=sr[0:1, ic * 128:(ic + 1) * 128],
                            rhs=ones11[:], start=True, stop=True)
                        rc = ep.tile([128, 1], F32)
                        nc.vector.reciprocal(out=rc[:], in_=oc[:, fout:fout + 1])
                        an = ep.tile([128, fout], BF16)
                        nc.vector.tensor_scalar_mul(an[:], oc[:, 0:fout], rc[:])
                        rp = ep.tile([128, fout], BF16)
                        nc.vector.tensor_scalar_max(rp[:], an[:], 0.0)
                        emt = ep.tile([128, fout], BF16)
                        nc.scalar.activation(out=emt[:], in_=an[:], func=AF.Exp)
                        mn = ep.tile([128, fout], BF16)
                        nc.vector.tensor_scalar(mn[:], emt[:], 1.0, -1.0,
                                                OP.min, OP.add)
                        nc.vector.tensor_add(helu[:, ic, :], mn[:], rp[:])
                        ex2 = ep.tile([128, fout], BF16)
                        nc.scalar.activation(out=ex2[:], in_=helu[:, ic, :],
                                             func=AF.Exp,
                                             accum_out=smv[:, ic:ic + 1])
                    lnv = consts.tile([128, rch], F32)
                    nc.scalar.activation(out=lnv[:], in_=smv[:], func=AF.Ln)
                    for ic in range(rch):
                        fin = ep.tile([128, fout], F32, name="fin")
                        nc.vector.tensor_scalar_sub(fin[:], helu[:, ic, :],
                                                    lnv[:, ic:ic + 1])
                        nc.sync.dma_start(out=out[ic * 128:(ic + 1) * 128, :], in_=fin[:])

    nc.compile()
    return nc


_CACHE = {}


def _get_programs():
    if "p1" not in _CACHE:
        _CACHE["p1"] = build_phase1()
        _CACHE["p2"] = build_phase2()
    return _CACHE["p1"], _CACHE["p2"]


def _pp_scalars(f2):
    """[N] f32 -> [128, N/128] f32 per-partition layout (p, c) = f[c*128+p]."""
    nch = f2.shape[0] // 128
    return np.ascontiguousarray(f2.reshape(nch, 128).T.astype(np.float32))


def make_in1(x, adj, W_heads, a1_heads, a2_heads):
    bf = ml_dtypes.bfloat16
    xT = np.ascontiguousarray(x.T).astype(bf)
    maskT = np.ascontiguousarray((adj > 0).T.astype(np.float32)).astype(bf)
    in1 = []
    for h in range(NCORES):
        f1 = x @ (W_heads[h] @ a1_heads[h])
        f2 = x @ (W_heads[h] @ a2_heads[h])
        in1.append({
            "xT": xT, "maskT": maskT,
            "wcat": np.ascontiguousarray(W_heads[h].astype(bf)),
            "wvec": np.exp((1.0 - ALPHA) * f1).astype(bf),
            "u2i": _pp_scalars(np.exp(f2)),
            "v2i": _pp_scalars(np.exp(ALPHA * f2)),
        })
    return in1, maskT


def make_in2(hT, maskT, W_out, a1_out, a2_out):
    bf = ml_dtypes.bfloat16
    h32 = hT.astype(np.float32).T           # [N, hfull]
    f1o = h32 @ (W_out @ a1_out)
    f2o = h32 @ (W_out @ a2_out)
    wocat = np.ascontiguousarray(W_out.astype(bf))
    u2o = _pp_scalars(np.exp(f2o))
    v2o = _pp_scalars(np.exp(ALPHA * f2o))
    wo_full = np.exp((1.0 - ALPHA) * f1o).astype(bf)
    rows = N // NCORES
    in2 = []
    for c in range(NCORES):
        in2.append({
            "hT": hT,
            "m2": np.ascontiguousarray(maskT[:, c * rows:(c + 1) * rows]),
            "wocat": wocat,
            "wrow": np.ascontiguousarray(wo_full[c * rows:(c + 1) * rows]),
            "u2i": u2o, "v2i": v2o,
        })
    return in2


def kernel(x, adj, W_heads, a1_heads, a2_heads, W_out, a1_out, a2_out, **_):
    x = np.asarray(x, dtype=np.float32)
    adj = np.asarray(adj)
    W_heads = np.asarray(W_heads, dtype=np.float32)
    a1_heads = np.asarray(a1_heads, dtype=np.float32)
    a2_heads = np.asarray(a2_heads, dtype=np.float32)
    W_out = np.asarray(W_out, dtype=np.float32)
    a1_out = np.asarray(a1_out, dtype=np.float32)
    a2_out = np.asarray(a2_out, dtype=np.float32)

    bf = ml_dtypes.bfloat16
    p1, p2 = _get_programs()

    in1, maskT = make_in1(x, adj, W_heads, a1_heads, a2_heads)
    r1 = run_bass_kernel_spmd(p1, in1, core_ids=list(range(NCORES))).results

    hT = np.concatenate([r1[h]["hout"] for h in range(NCORES)], axis=0)
    hT = np.ascontiguousarray(hT.astype(bf))

    in2 = make_in2(hT, maskT, W_out, a1_out, a2_out)
    r2 = run_bass_kernel_spmd(p2, in2, core_ids=list(range(NCORES))).results

    out = np.concatenate([r2[c]["out"] for c in range(NCORES)], axis=0)
    return out.astype(np.float32)


# revision 8
# speedup vs baseline: 1.4787x; 1.2306x over previous
"""GAT (2-layer, 8-head) Trainium2 kernel, 8-core SPMD.

Phase 1: head-parallel — core h computes head h's GAT layer over the full
  graph.  Uses the identity  exp(lrelu(f1_i + f2_j)) = v1_i * max(w_i*u2_j, v2_j)
  with w = exp((1-a)f1), u2 = exp(f2), v2 = exp(a*f2); the v1_i factor is a
  per-row scale that cancels in the softmax normalization, so the [N,N]
  unnormalized attention needs ONE 4x-mode tensor_scalar (mult+max against
  two per-partition scalars) and ONE 2x-mode tensor_tensor mask multiply per
  128-row tile (mask multiply split between DVE and GpSimd).  The tiny f1/f2
  vectors (x @ (W@a), O(N*F)) are folded on the host like the W@a folds, so
  the attention stream starts immediately.  A zero-weight dummy matmul gates
  each 4-tile block of PE work so the tensor engine runs in long bursts and
  ramps out of its low-power state.
Phase 2: row-parallel — host gathers h_T [512, N] (bf16), every core computes
  the full Wh2 = h@W_out chunk-by-chunk interleaved with its attention
  matmuls (keeps PE continuously busy), then elu + log_softmax (no max-shift;
  logits are small) for its own N/8-row slice.
"""

import sys

for p in ("/opt/trn_rl_repo", "/opt/pypackages"):
    if p not in sys.path:
        sys.path.append(p)

import numpy as np
import ml_dtypes

import concourse.bass as bass
import concourse.bacc as bacc
import concourse.tile as tile
from concourse import mybir
from concourse.bass_utils import run_bass_kernel_spmd
from concourse.masks import make_identity

BF16 = mybir.dt.bfloat16
F32 = mybir.dt.float32
AX = mybir.AxisListType
OP = mybir.AluOpType
AF = mybir.ActivationFunctionType

N, FIN, HID, HEADS, FOUT = 4096, 512, 64, 8, 256
NCORES = 8
ALPHA = 0.2


def _flat_write_ap(t, rows, cols):
    return bass.AP(tensor=t, offset=0, ap=[[cols, rows], [1, cols]])


def build_phase1(n=N, fin=FIN, hid=HID):
    """Per-core: xT [fin, n] bf16, maskT [n, n] bf16, wcat [fin, hid] bf16,
    wvec [n] bf16 (= exp((1-a)f1)), u2i/v2i [128, n/128] f32
    -> hout [hid, n] bf16 (elu'd, normalized head features, transposed)."""
    nc = bacc.Bacc("TRN2", target_bir_lowering=False, debug=False,
                   enable_asserts=False)
    kch = fin // 128          # contraction chunks for x@W
    nch = n // 128            # 128-row chunks of nodes
    nib = max(1, n // 512)    # 512-col i-blocks for PSUM banks
    ibw = min(n, 512)

    xT = nc.dram_tensor("xT", [fin, n], BF16, kind="ExternalInput")
    maskT = nc.dram_tensor("maskT", [n, n], BF16, kind="ExternalInput")
    wcat = nc.dram_tensor("wcat", [fin, hid], BF16, kind="ExternalInput")
    wvec = nc.dram_tensor("wvec", [n], BF16, kind="ExternalInput")
    u2i = nc.dram_tensor("u2i", [128, nch], F32, kind="ExternalInput")
    v2i = nc.dram_tensor("v2i", [128, nch], F32, kind="ExternalInput")
    hout = nc.dram_tensor("hout", [hid, n], BF16, kind="ExternalOutput")
    scrR = nc.dram_tensor("scrR", [n], BF16)
    scrRb = nc.dram_tensor("scrRb", [n], BF16)

    with tile.TileContext(nc) as tc:
        with tc.tile_pool(name="consts", bufs=1) as consts:
            wsb = consts.tile([128, kch, hid], BF16)
            for kc in range(kch):
                nc.sync.dma_start(out=wsb[:, kc, :], in_=wcat[kc * 128:(kc + 1) * 128, :])
            u2c = consts.tile([128, nch], F32)
            v2c = consts.tile([128, nch], F32)
            nc.sync.dma_start(out=u2c[:], in_=u2i[:, :])
            nc.sync.dma_start(out=v2c[:], in_=v2i[:, :])
            w1b = consts.tile([128, n], BF16)
            for q4 in range(4):
                qn = n // 4
                bap = bass.AP(tensor=wvec, offset=q4 * qn, ap=[[0, 128], [1, qn]])
                nc.sync.dma_start(out=w1b[:, q4 * qn:(q4 + 1) * qn], in_=bap)
            whall = consts.tile([128, nch, hid + 1], BF16)
            nc.vector.memset(whall[:, :, hid:hid + 1], 1.0)

            # ---- Wh = x @ W  (column-major loads so chunk 0 is ready fast) ----
            with tc.tile_pool(name="xpool", bufs=1) as xpool:
                xsb = xpool.tile([128, kch, n], BF16)
                for q8 in range(8):
                    sl = slice(q8 * (n // 8), (q8 + 1) * (n // 8))
                    for kc in range(kch):
                        nc.sync.dma_start(out=xsb[:, kc, sl],
                                          in_=xT[kc * 128:(kc + 1) * 128, sl])
                with tc.tile_pool(name="whps", bufs=4, space="PSUM") as whps:
                    for nch_i in range(nch):
                        pw = whps.tile([128, hid], F32)
                        for kc in range(kch):
                            nc.tensor.matmul(
                                out=pw[:],
                                lhsT=xsb[:, kc, nch_i * 128:(nch_i + 1) * 128],
                                rhs=wsb[:, kc, :],
                                start=(kc == 0), stop=(kc == kch - 1))
                        nc.scalar.activation(out=whall[:, nch_i, 0:hid],
                                             in_=pw[:], func=AF.Copy)

            outTb = consts.tile([hid + 1, n], BF16)

            # ---- attention: z = max(w1b*u2[j], v2[j]) * mask; two j-chunks
            # per iteration share one wide mask-multiply (halves the per-op
            # DVE fixed overhead) and the mask streams on the Act HWDGE
            # queue so it never queues behind the feature loads ----
            with (
                tc.tile_pool(name="mpool", bufs=3) as mpool,
                tc.tile_pool(name="ampool", bufs=2) as ampool,
                tc.tile_pool(name="ppool", bufs=3) as ppool,
                tc.tile_pool(name="atps", bufs=nib, space="PSUM") as atps,
            ):
                pss = [atps.tile([hid + 1, ibw], F32, name=f"pss{_i}", tag="pss")
                       for _i in range(nib)]
                for jp in range(nch // 2):
                    mt = mpool.tile([128, 2, n], BF16)
                    mx = ampool.tile([128, 2, n], BF16)
                    pt = ppool.tile([128, 2, n], BF16)
                    for k in range(2):
                        jc = 2 * jp + k
                        nc.scalar.dma_start(out=mt[:, k, :],
                                            in_=maskT[jc * 128:(jc + 1) * 128, :])
                        nc.vector.tensor_scalar(
                            mx[:, k, :], w1b[:], u2c[:, jc:jc + 1],
                            v2c[:, jc:jc + 1], OP.mult, OP.max)
                    nc.vector.tensor_mul(pt[:], mx[:], mt[:])
                    for k in range(2):
                        jc = 2 * jp + k
                        for ib in range(nib):
                            nc.tensor.matmul(
                                out=pss[ib][:],
                                lhsT=whall[:, jc, :],
                                rhs=pt[:, k, ib * ibw:(ib + 1) * ibw],
                                start=(jc == 0), stop=(jc == nch - 1))

                # ---- epilogue: normalize + elu, out h_T [hid, n] bf16 ----
                for ib in range(nib):
                    nc.scalar.activation(
                        out=outTb[:, ib * ibw:(ib + 1) * ibw],
                        in_=pss[ib][:], func=AF.Copy)
            # reciprocal of rowsums: bounce via DRAM to reshape the [1, n]
            # row onto 128 partitions (cheap DVE recip), then broadcast
            nc.sync.dma_start(out=_flat_write_ap(scrR, 1, n),
                              in_=outTb[hid:hid + 1, :])
            r128 = consts.tile([128, nch], BF16)
            nc.sync.dma_start(out=r128[:],
                              in_=bass.AP(tensor=scrR, offset=0,
                                          ap=[[nch, 128], [1, nch]]))
            rb128 = consts.tile([128, nch], BF16)
            with nc.allow_low_precision(reason="softmax denom reciprocal to bf16"):
                nc.vector.reciprocal(out=rb128[:], in_=r128[:])
            nc.sync.dma_start(out=_flat_write_ap(scrRb, 128, nch), in_=rb128[:])
            rsb = consts.tile([hid, n], BF16)
            for q4 in range(4):
                qn = n // 4
                bap = bass.AP(tensor=scrRb, offset=q4 * qn, ap=[[0, hid], [1, qn]])
                nc.sync.dma_start(out=rsb[:, q4 * qn:(q4 + 1) * qn], in_=bap)
            with tc.tile_pool(name="ep1", bufs=3) as ep1:
                hv = ep1.tile([hid, n], BF16, tag="ep1")
                nc.vector.tensor_mul(hv[:], outTb[0:hid, :], rsb[:])
                rp = ep1.tile([hid, n], BF16, tag="ep1", name="rp")
                nc.vector.tensor_scalar_max(rp[:], hv[:], 0.0)
                em = ep1.tile([hid, n], BF16, tag="ep1", name="em")
                nc.scalar.activation(out=em[:], in_=hv[:], func=AF.Exp)
                mn = ep1.tile([hid, n], BF16, tag="ep1", name="mn")
                nc.vector.tensor_scalar(mn[:], em[:], 1.0, -1.0, OP.min, OP.add)
                hsb = ep1.tile([hid, n], BF16, tag="ep1", name="hsb")
                nc.vector.tensor_add(hsb[:], mn[:], rp[:])
                for q4 in range(4):
                    sl = slice(q4 * (n // 4), (q4 + 1) * (n // 4))
                    nc.sync.dma_start(out=hout[:, sl], in_=hsb[:, sl])

    nc.compile()
    return nc


def build_phase2(n=N, hfull=HEADS * HID, fout=FOUT):
    """Per-core: hT [hfull, n] bf16, m2 [n, n/NCORES] bf16,
    wocat [hfull, fout] bf16, wrow [n/NCORES] bf16, u2i/v2i [128, n/128] f32
    -> out [n/NCORES, fout] f32 (log_softmax rows)."""
    nc = bacc.Bacc("TRN2", target_bir_lowering=False, debug=False,
                   enable_asserts=False)
    rows = n // NCORES        # rows this core owns
    kch = hfull // 128
    nch = n // 128
    rch = rows // 128         # output 128-row chunks
    fb = fout // 128          # 128-col blocks of fout

    hT = nc.dram_tensor("hT", [hfull, n], BF16, kind="ExternalInput")
    m2 = nc.dram_tensor("m2", [n, rows], BF16, kind="ExternalInput")
    wocat = nc.dram_tensor("wocat", [hfull, fout], BF16, kind="ExternalInput")
    wrow = nc.dram_tensor("wrow", [rows], BF16, kind="ExternalInput")
    u2i = nc.dram_tensor("u2i", [128, nch], F32, kind="ExternalInput")
    v2i = nc.dram_tensor("v2i", [128, nch], F32, kind="ExternalInput")
    out = nc.dram_tensor("out", [rows, fout], F32, kind="ExternalOutput")

    with tile.TileContext(nc) as tc:
        with tc.tile_pool(name="consts", bufs=1) as consts:
            id128 = consts.tile([128, 128], F32)
            make_identity(nc, id128[:])
            wosb = consts.tile([128, kch, fout], BF16)
            for kc in range(kch):
                nc.sync.dma_start(out=wosb[:, kc, :],
                                  in_=wocat[kc * 128:(kc + 1) * 128, :])
            u2c = consts.tile([128, nch], F32)
            v2c = consts.tile([128, nch], F32)
            nc.sync.dma_start(out=u2c[:], in_=u2i[:, :])
            nc.sync.dma_start(out=v2c[:], in_=v2i[:, :])
            w2b = consts.tile([128, rows], BF16)
            nc.sync.dma_start(out=w2b[:],
                              in_=bass.AP(tensor=wrow, offset=0,
                                          ap=[[0, 128], [1, rows]]))
            hsb = consts.tile([128, kch, n], BF16)
            for q8 in range(8):
                sl = slice(q8 * (n // 8), (q8 + 1) * (n // 8))
                for kc in range(kch):
                    nc.sync.dma_start(out=hsb[:, kc, sl],
                                      in_=hT[kc * 128:(kc + 1) * 128, sl])

            woall = consts.tile([128, nch, fout + 1], BF16)
            nc.vector.memset(woall[:, :, fout:fout + 1], 1.0)

            # ---- interleaved: Wh2 chunk jc, then attention tile jc ----
            with (
                tc.tile_pool(name="w2ps", bufs=2, space="PSUM") as w2ps,
                tc.tile_pool(name="m2pool", bufs=4) as m2pool,
                tc.tile_pool(name="am2pool", bufs=3) as am2pool,
                tc.tile_pool(name="p2pool", bufs=4) as p2pool,
                tc.tile_pool(name="sfpool", bufs=fb) as sfpool,
                tc.tile_pool(name="a2ps", bufs=fb, space="PSUM") as a2psf,
                tc.tile_pool(name="a2psr", bufs=1, space="PSUM") as a2psr,
            ):
                psf = [a2psf.tile([128, rows], F32, name=f"psf{_i}", tag="psf") for _i in range(fb)]
                psr = a2psr.tile([1, rows], F32)
                pts = {}

                def _attn(jc):
                    pt = pts.pop(jc)
                    for f in range(fb):
                        nc.tensor.matmul(
                            out=psf[f][:],
                            lhsT=woall[:, jc, f * 128:(f + 1) * 128],
                            rhs=pt[:],
                            start=(jc == 0), stop=(jc == nch - 1))
                    nc.tensor.matmul(
                        out=psr[:],
                        lhsT=woall[:, jc, fout:fout + 1],
                        rhs=pt[:],
                        start=(jc == 0), stop=(jc == nch - 1))

                for jc in range(nch):
                    pw = w2ps.tile([128, fout], F32)
                    for kc in range(kch):
                        nc.tensor.matmul(
                            out=pw[:],
                            lhsT=hsb[:, kc, jc * 128:(jc + 1) * 128],
                            rhs=wosb[:, kc, :],
                            start=(kc == 0), stop=(kc == kch - 1))
                    nc.scalar.activation(out=woall[:, jc, 0:fout],
                                         in_=pw[:], func=AF.Copy)

                    mt = m2pool.tile([128, rows], BF16)
                    nc.scalar.dma_start(out=mt[:], in_=m2[jc * 128:(jc + 1) * 128, :])
                    mx = am2pool.tile([128, rows], BF16)
                    nc.vector.tensor_scalar(
                        mx[:], w2b[:], u2c[:, jc:jc + 1], v2c[:, jc:jc + 1],
                        OP.mult, OP.max)
                    pt = p2pool.tile([128, rows], BF16)
                    nc.vector.tensor_mul(pt[:], mx[:], mt[:])
                    pts[jc] = pt
                    # attention for chunk jc-1: its woall scalar-copy overlaps
                    # chunk jc's Wh2 matmuls, so the PE stream never bubbles
                    if jc >= 1:
                        _attn(jc - 1)
                _attn(nch - 1)

                sf = [sfpool.tile([128, rows], F32, name=f"sf{_i}", tag="sf") for _i in range(fb)]
                sr = consts.tile([1, rows], F32)
                for f in range(fb):
                    nc.vector.tensor_copy(out=sf[f][:], in_=psf[f][:])
                nc.vector.tensor_copy(out=sr[:], in_=psr[:])

                # ---- epilogue: per 128-row chunk transpose, norm, elu;
                # log_softmax without max-shift (logits are small) and with
                # Exp/Ln batched to avoid act-table thrash ----
                ones11 = consts.tile([1, 1], F32)
                nc.vector.memset(ones11[:], 1.0)
                helu = consts.tile([128, rch, fout], BF16)
                smv = consts.tile([128, rch], F32)
                with (
                    tc.tile_pool(name="ocps", bufs=2, space="PSUM") as ocps,
                    tc.tile_pool(name="ep", bufs=3) as ep,
                ):
                    for ic in range(rch):
                        oc = ocps.tile([128, fout + 1], F32)
                        for f in range(fb):
                            nc.tensor.transpose(
                                out=oc[:, f * 128:(f + 1) * 128],
                                in_=sf[f][:, ic * 128:(ic + 1) * 128],
                                identity=id128[:])
                        nc.tensor.matmul(
                            out=oc[:, fout:fout + 1],
                            lhsT=sr[0:1, ic * 128:(ic + 1) * 128],
                            rhs=ones11[:], start=True, stop=True)
                        rc = ep.tile([128, 1], F32)
                        nc.vector.reciprocal(out=rc[:], in_=oc[:, fout:fout + 1])
                        an = ep.tile([128, fout], BF16)
                        nc.vector.tensor_scalar_mul(an[:], oc[:, 0:fout], rc[:])
                        rp = ep.tile([128, fout], BF16)
                        nc.vector.tensor_scalar_max(rp[:], an[:], 0.0)
                        emt = ep.tile([128, fout], BF16)
                        nc.scalar.activation(out=emt[:], in_=an[:], func=AF.Exp)
                        mn = ep.tile([128, fout], BF16)
                        nc.vector.tensor_scalar(mn[:], emt[:], 1.0, -1.0,
                                                OP.min, OP.add)
                        nc.vector.tensor_add(helu[:, ic, :], mn[:], rp[:])
                        ex2 = ep.tile([128, fout], BF16)
                        nc.scalar.activation(out=ex2[:], in_=helu[:, ic, :],
                                             func=AF.Exp,
                                             accum_out=smv[:, ic:ic + 1])
                    lnv = consts.tile([128, rch], F32)
                    nc.scalar.activation(out=lnv[:], in_=smv[:], func=AF.Ln)
                    for ic in range(rch):
                        fin = ep.tile([128, fout], F32, name="fin")
                        nc.vector.tensor_scalar_sub(fin[:], helu[:, ic, :],
                                                    lnv[:, ic:ic + 1])
                        nc.sync.dma_start(out=out[ic * 128:(ic + 1) * 128, :], in_=fin[:])

    nc.compile()
    return nc


_CACHE = {}


def _get_programs():
    if "p1" not in _CACHE:
        _CACHE["p1"] = build_phase1()
        _CACHE["p2"] = build_phase2()
    return _CACHE["p1"], _CACHE["p2"]


def _pp_scalars(f2):
    """[N] f32 -> [128, N/128] f32 per-partition layout (p, c) = f[c*128+p]."""
    nch = f2.shape[0] // 128
    return np.ascontiguousarray(f2.reshape(nch, 128).T.astype(np.float32))


def make_in1(x, adj, W_heads, a1_heads, a2_heads):
    bf = ml_dtypes.bfloat16
    xT = np.ascontiguousarray(x.T).astype(bf)
    maskT = np.ascontiguousarray((adj > 0).T.astype(np.float32)).astype(bf)
    in1 = []
    for h in range(NCORES):
        f1 = x @ (W_heads[h] @ a1_heads[h])
        f2 = x @ (W_heads[h] @ a2_heads[h])
        in1.append({
            "xT": xT, "maskT": maskT,
            "wcat": np.ascontiguousarray(W_heads[h].astype(bf)),
            "wvec": np.exp((1.0 - ALPHA) * f1).astype(bf),
            "u2i": _pp_scalars(np.exp(f2)),
            "v2i": _pp_scalars(np.exp(ALPHA * f2)),
        })
    return in1, maskT


def make_in2(hT, maskT, W_out, a1_out, a2_out):
    bf = ml_dtypes.bfloat16
    h32 = hT.astype(np.float32).T           # [N, hfull]
    f1o = h32 @ (W_out @ a1_out)
    f2o = h32 @ (W_out @ a2_out)
    wocat = np.ascontiguousarray(W_out.astype(bf))
    u2o = _pp_scalars(np.exp(f2o))
    v2o = _pp_scalars(np.exp(ALPHA * f2o))
    wo_full = np.exp((1.0 - ALPHA) * f1o).astype(bf)
    rows = N // NCORES
    in2 = []
    for c in range(NCORES):
        in2.append({
            "hT": hT,
            "m2": np.ascontiguousarray(maskT[:, c * rows:(c + 1) * rows]),
            "wocat": wocat,
            "wrow": np.ascontiguousarray(wo_full[c * rows:(c + 1) * rows]),
            "u2i": u2o, "v2i": v2o,
        })
    return in2


def kernel(x, adj, W_heads, a1_heads, a2_heads, W_out, a1_out, a2_out, **_):
    x = np.asarray(x, dtype=np.float32)
    adj = np.asarray(adj)
    W_heads = np.asarray(W_heads, dtype=np.float32)
    a1_heads = np.asarray(a1_heads, dtype=np.float32)
    a2_heads = np.asarray(a2_heads, dtype=np.float32)
    W_out = np.asarray(W_out, dtype=np.float32)
    a1_out = np.asarray(a1_out, dtype=np.float32)
    a2_out = np.asarray(a2_out, dtype=np.float32)

    bf = ml_dtypes.bfloat16
    p1, p2 = _get_programs()

    in1, maskT = make_in1(x, adj, W_heads, a1_heads, a2_heads)
    r1 = run_bass_kernel_spmd(p1, in1, core_ids=list(range(NCORES))).results

    hT = np.concatenate([r1[h]["hout"] for h in range(NCORES)], axis=0)
    hT = np.ascontiguousarray(hT.astype(bf))

    in2 = make_in2(hT, maskT, W_out, a1_out, a2_out)
    r2 = run_bass_kernel_spmd(p2, in2, core_ids=list(range(NCORES))).results

    out = np.concatenate([r2[c]["out"] for c in range(NCORES)], axis=0)
    return out.astype(np.float32)
